# revision 1
# baseline (speedup 1.0000x reference)
"""Bass/Trainium2 kernel for nn_AttentionLayer (B=8, SQ=SV=2048, D=1024, fp32).

attention = softmax(Q @ V^T) @ V, per batch element.

Strategy
--------
- Batch-parallel over 8 NeuronCores (1 batch element per core, no collectives).
- Per core: fp16 operands on TensorE (full rate, 10-bit mantissa keeps
  scale-relative absmax error ~1e-2 vs the fp32 reference), fp32 PSUM
  accumulation, softmax in fp32 (DVE row-max, ScalarE exp LUT with per-row
  bias and fused row-sum accum_out).
- Layout: the scores matmul needs d-major Q^T/V^T operands and the second
  matmul needs P^T; all transposes ride the DMA xbar (2-byte dtype).
  fp32->fp16 casts are contiguous SWDGE casting DMAs into DRAM scratch;
  transposed loads are batched (>=512 source rows per DMA_TRANSPOSE), all
  on the Sync engine (concurrent xposes from both HWDGE engines corrupt
  data; the xpose also executes synchronously on the issuing engine).
- mm1 iterates contraction-outer so 4 consecutive matmuls share weights;
  all matmuls are converted to self-loading (explicit LDWEIGHTS stripped)
  which lets consecutive matmuls pipeline at full rate (~216 ns/MM).
- Software pipeline: stage2 (E @ V) for q-tile i is emitted alongside
  stage1 of q-tile i+G, so the PE never waits on the E DRAM round trip.
"""

import sys

if "/opt/trn_rl_repo" not in sys.path:
    sys.path.insert(0, "/opt/trn_rl_repo")

import numpy as np

B, SQ, SV, D = 8, 2048, 2048, 1024
P = 128
N_CORES = 8


def _strip_all_ldweights(nc):
    """Remove every InstLdweights, migrating its semaphore waits/updates onto
    the next PE instruction (its paired InstMatmult). Leaves self-loading
    matmuls that walrus --enable-ldw-opt=true can schedule with
    background-buffer weight loads."""
    import concourse.mybir as mybir

    removed = 0
    for fn in nc.m.functions:
        for bb in fn.blocks:
            keep = []
            pending = []  # sync_infos from dropped LDWs awaiting the next MM
            for inst in bb.instructions:
                if isinstance(inst, mybir.InstLdweights):
                    if inst.sync_info is not None:
                        pending.append(inst.sync_info)
                    removed += 1
                    continue
                if isinstance(inst, mybir.InstMatmult):
                    inst.ldweights = True
                    if pending:
                        waits, updates = [], []
                        for si in pending:
                            waits.extend(list(si.on_wait))
                            updates.extend(list(si.on_update))
                        mi = inst.sync_info
                        if mi is not None:
                            waits.extend(list(mi.on_wait))
                            updates.extend(list(mi.on_update))
                        inst.sync_info = mybir.SyncInfo(
                            on_wait=waits, on_update=updates
                        )
                        pending = []
                keep.append(inst)
            assert not pending, "dangling LDW sync with no following matmul"
            bb.instructions[:] = keep
    return removed


def build_attention_nc(sq=SQ, sv=SV, d=D, lag=8, group=4):
    import concourse.bass as bass
    import concourse.mybir as mybir
    from concourse import bacc
    from concourse.tile import TileContext

    f32 = mybir.dt.float32
    f16 = mybir.dt.float16
    X = mybir.AxisListType.X
    Exp = mybir.ActivationFunctionType.Exp

    NQT, NST, NKT = sq // P, sv // P, d // P
    SCH = min(512, sv)   # scores psum chunk (one bank)
    NSCH = sv // SCH
    DCH = min(512, d)    # output psum chunk
    NDCH = d // DCH
    G = group
    NG = NQT // G
    assert NQT % G == 0 and lag % G == 0
    RBV = min(1024, sv)  # cast/xpose row-block
    RBQ = min(1024, sq)
    NRB_V, NRB_Q = sv // RBV, sq // RBQ

    nc = bacc.Bacc("TRN2", target_bir_lowering=False, debug=False)
    q = nc.dram_tensor("q", [sq, d], f32, kind="ExternalInput").ap()
    v = nc.dram_tensor("v", [sv, d], f32, kind="ExternalInput").ap()
    out = nc.dram_tensor("out", [sq, d], f32, kind="ExternalOutput").ap()

    # All xposes on the Sync engine: the xbar transpose path is shared
    # hardware — concurrent transposes from both HWDGE engines corrupt data.
    def xpose(out_ap, in_ap):
        nc.sync.dma_start_transpose(out_ap, in_ap)

    with TileContext(nc) as tc:
        with (
            tc.tile_pool(name="dram", bufs=1, space="DRAM") as dram_pool,
            tc.tile_pool(name="ehp", bufs=lag // group + 3, space="DRAM") as eh_pool,
            tc.tile_pool(name="resident", bufs=1) as res_pool,
            tc.tile_pool(name="ssb", bufs=2) as ssb_pool,
            tc.tile_pool(name="esb", bufs=3) as esb_pool,
            tc.tile_pool(name="etp", bufs=3) as et_pool,
            tc.tile_pool(name="osb", bufs=2) as osb_pool,
            tc.tile_pool(name="small", bufs=2 * (lag + 3)) as small_pool,
            tc.tile_pool(name="spsum", bufs=1, space="PSUM") as spsum_pool,
            tc.tile_pool(name="opsum", bufs=2, space="PSUM") as opsum_pool,
        ):
            # ---------------- stage A: fp16 casts + resident operands -------
            qh = dram_pool.tile([sq, d], f16, name="qh")
            vh = dram_pool.tile([sv, d], f16, name="vh")

            # Contiguous row-block casting DMAs (SWDGE): fp32 -> fp16 DRAM.
            for b in range(NRB_V):
                rs = slice(b * RBV, (b + 1) * RBV)
                nc.gpsimd.dma_start(out=vh[rs, :], in_=v[rs, :])
            for b in range(NRB_Q):
                rs = slice(b * RBQ, (b + 1) * RBQ)
                nc.gpsimd.dma_start(out=qh[rs, :], in_=q[rs, :])

            # V natural fp16 (rhs of mm2), straight casting DMAs into SBUF.
            vf = []
            for si in range(NST):
                t = res_pool.tile([P, d], f16, name=f"vf{si}")
                nc.gpsimd.dma_start(out=t, in_=v[si * P : (si + 1) * P, :])
                vf.append(t)

            # Transposed residents: per d-tile stripes of V^T and Q^T,
            # xposed in row blocks, emitted dk-interleaved so mm1 can start
            # as soon as the dk=0 stripes land.
            vT = [res_pool.tile([P, sv], f16, name=f"vT{dk}") for dk in range(NKT)]
            qT = [res_pool.tile([P, sq], f16, name=f"qT{dk}") for dk in range(NKT)]
            for dk in range(NKT):
                cs = slice(dk * P, (dk + 1) * P)
                for b in range(NRB_V):
                    rs = slice(b * RBV, (b + 1) * RBV)
                    xpose(vT[dk][:, rs], vh[rs, cs])
                for b in range(NRB_Q):
                    rs = slice(b * RBQ, (b + 1) * RBQ)
                    xpose(qT[dk][:, rs], qh[rs, cs])

            state = {}
            eh_group = [None] * NG

            def stage1(qi):
                # S = Q[qi] @ V^T -> softmax pieces -> E(fp16) -> DRAM group
                sp = [
                    spsum_pool.tile([P, SCH], f32, name=f"spsum{j}")
                    for j in range(NSCH)
                ]
                for dk in range(NKT):
                    lw = qT[dk][:, qi * P : (qi + 1) * P]
                    for j in range(NSCH):
                        nc.tensor.matmul(
                            sp[j],
                            lhsT=lw,
                            rhs=vT[dk][:, j * SCH : (j + 1) * SCH],
                            start=(dk == 0),
                            stop=(dk == NKT - 1),
                        )
                s_sb = ssb_pool.tile([P, sv], f32, name="s_sb")
                for j in range(NSCH):
                    nc.scalar.copy(s_sb[:, j * SCH : (j + 1) * SCH], sp[j])
                negm = small_pool.tile([P, 1], f32, name="negm")
                nc.vector.reduce_max(negm, s_sb, axis=X, negate=True)
                e_sb = esb_pool.tile([P, sv], f16, name="e_sb")
                lsum = small_pool.tile([P, 1], f32, name="lsum")
                nc.scalar.activation(
                    e_sb, s_sb, Exp, bias=negm, scale=1.0, accum_out=lsum
                )
                r = small_pool.tile([P, 1], f32, name="r")
                nc.vector.reciprocal(r, lsum)
                g, gi = qi // G, qi % G
                if eh_group[g] is None:
                    eh_group[g] = eh_pool.tile([G * P, sv], f16, name="ehg")
                nc.gpsimd.dma_start(
                    out=eh_group[g][gi * P : (gi + 1) * P, :], in_=e_sb
                )
                state[qi] = r

            def emit_group_xpose(g):
                # E^T stripes for the whole q-group: [128 s, G*128 q] per s-tile
                eT = et_pool.tile([P, NST, G * P], f16, name="eT")
                for sk in range(NST):
                    xpose(eT[:, sk, :], eh_group[g][:, sk * P : (sk + 1) * P])
                state[("eT", g)] = eT

            def stage2(qi):
                # out[qi] = (E @ V) * r
                r = state.pop(qi)
                g, gi = qi // G, qi % G
                eT = state[("eT", g)]
                qs = slice(gi * P, (gi + 1) * P)
                op = opsum_pool.tile([P, d], f32, name="opsum")
                for sk in range(NST):
                    for c in range(NDCH):
                        cs = slice(c * DCH, (c + 1) * DCH)
                        nc.tensor.matmul(
                            op[:, cs],
                            lhsT=eT[:, sk, qs],
                            rhs=vf[sk][:, cs],
                            start=(sk == 0),
                            stop=(sk == NST - 1),
                        )
                o_sb = osb_pool.tile([P, d], f32, name="o_sb")
                nc.vector.tensor_scalar_mul(o_sb, op, r)
                nc.gpsimd.dma_start(out=out[qi * P : (qi + 1) * P, :], in_=o_sb)
                if gi == G - 1:
                    state.pop(("eT", g))
                    eh_group[g] = None

            for qi in range(NQT + lag):
                if qi < NQT:
                    stage1(qi)
                    if qi % G == G - 1:
                        emit_group_xpose(qi // G)
                if qi >= lag:
                    stage2(qi - lag)

    import os

    if bool(int(os.environ.get("KERNEL_SELF_LDW", "1"))):
        _strip_all_ldweights(nc)
    nc.compile()
    return nc


_CACHE = {}


def _get_nc():
    if "nc" not in _CACHE:
        _CACHE["nc"] = build_attention_nc()
    return _CACHE["nc"]


def _install_trace_support():
    """Synthesize the antenv.axon_hooks module (absent in this image) and
    register the NTFF profile hook + disable the S3 artifact upload."""
    import types
    import antenv

    if "antenv.axon_hooks" not in sys.modules:
        mod = types.ModuleType("antenv.axon_hooks")
        mod._hook = None

        def set_axon_ntff_profile_hook(h):
            mod._hook = h

        def get_axon_ntff_profile_hook():
            return mod._hook

        mod.set_axon_ntff_profile_hook = set_axon_ntff_profile_hook
        mod.get_axon_ntff_profile_hook = get_axon_ntff_profile_hook
        sys.modules["antenv.axon_hooks"] = mod
        antenv.axon_hooks = mod

    mod = sys.modules["antenv.axon_hooks"]
    if mod._hook is None:
        from trn_agent_boot.trn_boot import _ntff_profile_via_ctypes

        mod._hook = _ntff_profile_via_ctypes("/opt/axon/libaxon_pjrt.so")

    import concourse.bass_utils as bu

    bu.upload_artifacts = lambda tmpdir: tmpdir


def _enable_walrus_ldw_opt():
    """Rewrite --enable-ldw-opt=false -> true in walrus_driver invocations.
    The walrus LDW optimization software-pipelines weight loads into the
    PE background buffer, hiding LDWEIGHTS behind running matmuls."""
    import concourse.bass_utils as bu

    if getattr(bu, "_ldw_opt_patched", False):
        return
    orig = bu.run_command

    def patched(argv, **kw):
        argv = [
            "--enable-ldw-opt=true" if a == "--enable-ldw-opt=false" else a
            for a in argv
        ]
        return orig(argv, **kw)

    bu.run_command = patched
    bu._ldw_opt_patched = True


def kernel(query: np.ndarray, value: np.ndarray) -> np.ndarray:
    from concourse.bass_utils import run_bass_kernel_spmd
    import os

    if bool(int(os.environ.get("KERNEL_LDW_OPT", "0"))):
        _enable_walrus_ldw_opt()

    assert query.shape == (B, SQ, D) and value.shape == (B, SV, D)
    nc = _get_nc()
    in_maps = [
        {
            "q": np.ascontiguousarray(query[b], dtype=np.float32),
            "v": np.ascontiguousarray(value[b], dtype=np.float32),
        }
        for b in range(N_CORES)
    ]
    trace = bool(int(os.environ.get("KERNEL_TRACE", "0")))
    kwargs = {}
    if trace:
        _install_trace_support()
        tdir = os.environ.get("KERNEL_TRACE_DIR")
        if tdir:
            os.makedirs(tdir, exist_ok=True)
            kwargs["tmpdir"] = tdir
    res = run_bass_kernel_spmd(
        nc, in_maps, core_ids=list(range(N_CORES)), trace=trace, **kwargs
    )
    if trace:
        _CACHE["last_results"] = res
    return np.stack([res.results[b]["out"] for b in range(N_CORES)], axis=0)



# revision 2
# speedup vs baseline: 1.4012x; 1.4012x over previous
"""Bass/Trainium2 kernel for nn_AttentionLayer (B=8, SQ=SV=2048, D=1024, fp32).

attention = softmax(Q @ V^T) @ V, per batch element.

Strategy (v2)
-------------
- Batch-parallel over 8 NeuronCores (1 batch element per core, no collectives).
- fp16 operands on TensorE (full rate), fp32 PSUM accumulation, softmax in
  fp32 (DVE row-max, ScalarE exp LUT with per-row bias + fused row-sum).
- Input staging redesigned vs v1 to start the PE early:
  * V is cast fp32->fp16 straight into SBUF (vf_all) in 4 row-wave SWDGE
    casts; vh (fp16 DRAM image) is then written back from SBUF on the
    scalar HWDGE ring (V fp32 is read from HBM exactly once).
  * V^T/Q^T stripes are produced by ONE multi-stripe DMA_TRANSPOSE call
    per 512-row chunk ([512,1024] DRAM -> [128,8,512] SBUF), 8 calls
    total instead of 32.
  * E (softmax numerator) is transposed SBUF->SBUF per q-tile
    ([128,2048] -> [128,16,128]) - no DRAM round trip for E at all.
- stage1 is emitted in (q-tile, s-chunk) units of 8 matmuls, interleaved
  so the first chunks of V^T/Q^T are consumed as soon as they land while
  later chunks still stream in.
- All transposes ride the sync-engine HWDGE ring exclusively (concurrent
  xposes from both HWDGE rings corrupt data; xbar path is shared).
- All matmuls converted to self-loading (explicit LDWEIGHTS stripped) so
  consecutive matmuls pipeline at full rate (~216 ns/MM for N=512).
"""

import sys

if "/opt/trn_rl_repo" not in sys.path:
    sys.path.insert(0, "/opt/trn_rl_repo")

import numpy as np

B, SQ, SV, D = 8, 2048, 2048, 1024
P = 128
N_CORES = 8


def _strip_all_ldweights(nc):
    """Remove every InstLdweights, migrating its semaphore waits/updates onto
    the next PE instruction (its paired InstMatmult). Leaves self-loading
    matmuls."""
    import concourse.mybir as mybir

    removed = 0
    for fn in nc.m.functions:
        for bb in fn.blocks:
            keep = []
            pending = []
            for inst in bb.instructions:
                if isinstance(inst, mybir.InstLdweights):
                    if inst.sync_info is not None:
                        pending.append(inst.sync_info)
                    removed += 1
                    continue
                if isinstance(inst, mybir.InstMatmult):
                    inst.ldweights = True
                    if pending:
                        waits, updates = [], []
                        for si in pending:
                            waits.extend(list(si.on_wait))
                            updates.extend(list(si.on_update))
                        mi = inst.sync_info
                        if mi is not None:
                            waits.extend(list(mi.on_wait))
                            updates.extend(list(mi.on_update))
                        inst.sync_info = mybir.SyncInfo(
                            on_wait=waits, on_update=updates
                        )
                        pending = []
                keep.append(inst)
            assert not pending, "dangling LDW sync with no following matmul"
            bb.instructions[:] = keep
    return removed


def build_attention_nc(sq=SQ, sv=SV, d=D, lag=6, rnd=4):
    import concourse.bass as bass
    import concourse.mybir as mybir
    from concourse import bacc
    from concourse.tile import TileContext

    f32 = mybir.dt.float32
    f16 = mybir.dt.float16
    X = mybir.AxisListType.X
    Exp = mybir.ActivationFunctionType.Exp

    NQT, NST, NKT = sq // P, sv // P, d // P
    CH = 512                  # row chunk (cast/xpose granularity = psum chunk)
    NCH = sv // CH            # 4
    NJ = sv // CH             # score chunks per q-tile = 4
    DCH = 512
    NDCH = d // DCH

    nc = bacc.Bacc("TRN2", target_bir_lowering=False, debug=False)
    q = nc.dram_tensor("q", [sq, d], f32, kind="ExternalInput").ap()
    v = nc.dram_tensor("v", [sv, d], f32, kind="ExternalInput").ap()
    out = nc.dram_tensor("out", [sq, d], f32, kind="ExternalOutput").ap()

    with TileContext(nc) as tc:
        with (
            tc.tile_pool(name="dram", bufs=1, space="DRAM") as dram_pool,
            tc.tile_pool(name="resident", bufs=1) as res_pool,
            tc.tile_pool(name="ssb", bufs=rnd + 1) as ssb_pool,
            tc.tile_pool(name="esb", bufs=2) as esb_pool,
            tc.tile_pool(name="etp", bufs=lag + 2) as et_pool,
            tc.tile_pool(name="osb", bufs=2) as osb_pool,
            tc.tile_pool(name="small", bufs=3 * (lag + 3)) as small_pool,
            tc.tile_pool(name="spsum", bufs=4, space="PSUM") as spsum_pool,
            tc.tile_pool(name="opsum", bufs=2, space="PSUM") as opsum_pool,
        ):
            # fp16 DRAM images (transpose sources)
            vh = dram_pool.tile([sv, d], f16, name="vh")
            qh = dram_pool.tile([sq, d], f16, name="qh")

            # Residents:
            # vf_all[p, si, d]  : V natural fp16 (mm2 rhs; also vh source)
            # vT_all[p, c, dk, s]: V^T stripes, chunk-major
            # qT_all[p, c, dk, r]: Q^T stripes, chunk-major
            vf_all = res_pool.tile([P, NST, d], f16, name="vf_all")
            vT_all = res_pool.tile([P, NCH, NKT, CH], f16, name="vT_all")
            qT_all = res_pool.tile([P, NCH, NKT, CH], f16, name="qT_all")

            TPC = CH // P  # tiles per chunk (4)

            # ---- SWDGE ring (gpsimd): V wave casts + Q chunk casts --------
            # Order chosen so V (needed in full early) and the first Q chunk
            # lead; later Q chunks trail behind later V waves.
            def v_wave(c):
                src = v[c * CH : (c + 1) * CH, :].rearrange(
                    "(si p) d -> p si d", p=P
                )
                nc.gpsimd.dma_start(
                    out=vf_all[:, c * TPC : (c + 1) * TPC, :], in_=src
                )

            def q_cast(c):
                nc.gpsimd.dma_start(
                    out=qh[c * CH : (c + 1) * CH, :],
                    in_=q[c * CH : (c + 1) * CH, :],
                )

            v_wave(0)
            q_cast(0)
            v_wave(1)
            q_cast(1)
            v_wave(2)
            q_cast(2)
            v_wave(3)
            q_cast(3)

            # ---- scalar HWDGE ring: vh writes (per wave) ------------------
            for c in range(NCH):
                dst = vh[c * CH : (c + 1) * CH, :].rearrange(
                    "(si p) d -> p si d", p=P
                )
                nc.scalar.dma_start(
                    out=dst, in_=vf_all[:, c * TPC : (c + 1) * TPC, :]
                )

            # ---- sync ring: all transposes --------------------------------
            # One multi-stripe call per chunk: [CH, d] -> [P, NKT, CH].
            def xpose_chunk(dst_all, src_dram, c):
                nc.sync.dma_start_transpose(
                    dst_all[:, c, :, :], src_dram[c * CH : (c + 1) * CH, :]
                )

            xpose_chunk(qT_all, qh, 0)
            for c in range(NCH):
                xpose_chunk(vT_all, vh, c)
            # qT chunks 1..3 are emitted later (between eT xposes) so they
            # don't head-of-line block the sync ring while qc1..3 finish.

            # ---------------- main pipeline --------------------------------
            s_sb = {}
            state = {}
            done = []
            emitted2 = 0
            qt_emitted = 1

            def unit(qi, j):
                # scores chunk: S[qi, j*CH:(j+1)*CH] += sum_dk QT.T VT
                sp = spsum_pool.tile([P, CH], f32, name="spsum")
                cq, rq = qi // TPC, qi % TPC
                for dk in range(NKT):
                    nc.tensor.matmul(
                        sp,
                        lhsT=qT_all[:, cq, dk, rq * P : (rq + 1) * P],
                        rhs=vT_all[:, j, dk, :],
                        start=(dk == 0),
                        stop=(dk == NKT - 1),
                    )
                if qi not in s_sb:
                    s_sb[qi] = ssb_pool.tile([P, sv], f32, name="s_sb")
                dst = s_sb[qi][:, j * CH : (j + 1) * CH]
                # balance PSUM evacuation between ScalarE and DVE
                if j % 2 == 0:
                    nc.scalar.copy(dst, sp)
                else:
                    nc.vector.tensor_copy(out=dst, in_=sp)

            def finish_stage1(qi):
                s = s_sb.pop(qi)
                negm = small_pool.tile([P, 1], f32, name="negm")
                nc.vector.reduce_max(negm, s, axis=X, negate=True)
                e_sb = esb_pool.tile([P, sv], f16, name="e_sb")
                lsum = small_pool.tile([P, 1], f32, name="lsum")
                nc.scalar.activation(
                    e_sb, s, Exp, bias=negm, scale=1.0, accum_out=lsum
                )
                r = small_pool.tile([P, 1], f32, name="r")
                nc.vector.reciprocal(r, lsum)
                # E^T for this q-tile, SBUF -> SBUF (no DRAM round trip)
                eT = et_pool.tile([P, NST, P], f16, name="eT")
                nc.sync.dma_start_transpose(eT, e_sb)
                state[qi] = (r, eT)

            def stage2(qi):
                r, eT = state.pop(qi)
                op = opsum_pool.tile([P, d], f32, name="opsum")
                for sk in range(NST):
                    for c2 in range(NDCH):
                        cs = slice(c2 * DCH, (c2 + 1) * DCH)
                        nc.tensor.matmul(
                            op[:, cs],
                            lhsT=eT[:, sk, :],
                            rhs=vf_all[:, sk, cs],
                            start=(sk == 0),
                            stop=(sk == NST - 1),
                        )
                o_sb = osb_pool.tile([P, d], f32, name="o_sb")
                nc.vector.tensor_scalar_mul(o_sb, op, r)
                nc.scalar.dma_start(
                    out=out[qi * P : (qi + 1) * P, :], in_=o_sb
                )

            ucount = {qi: 0 for qi in range(NQT)}
            order = []
            for r0 in range(0, NQT, rnd):
                for j in range(NJ):
                    for qi in range(r0, r0 + rnd):
                        order.append((qi, j))

            for qi, j in order:
                unit(qi, j)
                ucount[qi] += 1
                if ucount[qi] == NJ:
                    finish_stage1(qi)
                    done.append(qi)
                    # emit the next qT chunk xpose between eT xposes so the
                    # sync ring never blocks on an unfinished q cast
                    if qt_emitted < NCH and len(done) >= 2 * qt_emitted:
                        xpose_chunk(qT_all, qh, qt_emitted)
                        qt_emitted += 1
                    while len(done) - emitted2 > lag:
                        stage2(done[emitted2])
                        emitted2 += 1
            while qt_emitted < NCH:
                xpose_chunk(qT_all, qh, qt_emitted)
                qt_emitted += 1
            while emitted2 < len(done):
                stage2(done[emitted2])
                emitted2 += 1

    import os

    if bool(int(os.environ.get("KERNEL_SELF_LDW", "1"))):
        _strip_all_ldweights(nc)
    nc.compile()
    return nc


_CACHE = {}


def _get_nc():
    if "nc" not in _CACHE:
        _CACHE["nc"] = build_attention_nc()
    return _CACHE["nc"]


def _install_trace_support():
    """Synthesize the antenv.axon_hooks module (absent in this image) and
    register the NTFF profile hook + disable the S3 artifact upload."""
    import types
    import antenv

    if "antenv.axon_hooks" not in sys.modules:
        mod = types.ModuleType("antenv.axon_hooks")
        mod._hook = None

        def set_axon_ntff_profile_hook(h):
            mod._hook = h

        def get_axon_ntff_profile_hook():
            return mod._hook

        mod.set_axon_ntff_profile_hook = set_axon_ntff_profile_hook
        mod.get_axon_ntff_profile_hook = get_axon_ntff_profile_hook
        sys.modules["antenv.axon_hooks"] = mod
        antenv.axon_hooks = mod

    mod = sys.modules["antenv.axon_hooks"]
    if mod._hook is None:
        from trn_agent_boot.trn_boot import _ntff_profile_via_ctypes

        mod._hook = _ntff_profile_via_ctypes("/opt/axon/libaxon_pjrt.so")

    import concourse.bass_utils as bu

    bu.upload_artifacts = lambda tmpdir: tmpdir


def kernel(query: np.ndarray, value: np.ndarray) -> np.ndarray:
    from concourse.bass_utils import run_bass_kernel_spmd
    import os

    assert query.shape == (B, SQ, D) and value.shape == (B, SV, D)
    nc = _get_nc()
    in_maps = [
        {
            "q": np.ascontiguousarray(query[b], dtype=np.float32),
            "v": np.ascontiguousarray(value[b], dtype=np.float32),
        }
        for b in range(N_CORES)
    ]
    trace = bool(int(os.environ.get("KERNEL_TRACE", "0")))
    kwargs = {}
    if trace:
        _install_trace_support()
        tdir = os.environ.get("KERNEL_TRACE_DIR")
        if tdir:
            os.makedirs(tdir, exist_ok=True)
            kwargs["tmpdir"] = tdir
    res = run_bass_kernel_spmd(
        nc, in_maps, core_ids=list(range(N_CORES)), trace=trace, **kwargs
    )
    if trace:
        _CACHE["last_results"] = res
    return np.stack([res.results[b]["out"] for b in range(N_CORES)], axis=0)


# revision 6
# speedup vs baseline: 1.5691x; 1.1198x over previous
"""Bass/Trainium2 kernel for nn_AttentionLayer (B=8, SQ=SV=2048, D=1024, fp32).

attention = softmax(Q @ V^T) @ V, per batch element.

Strategy (v3)
-------------
- Batch-parallel over 8 NeuronCores (1 batch element per core, no collectives).
- fp16 operands on TensorE (full rate), fp32 PSUM accumulation, softmax in
  fp32 (DVE row-max, ScalarE exp LUT with per-row bias + fused row-sum).
- Input staging minimizes prologue HBM traffic (the v2 bottleneck):
  * V: 4 SWDGE wave casts fp32->fp16 straight into SBUF (8MB HBM read,
    nothing else). V^T stripes are produced by PE-mode transposes
    (tensor.transpose via identity) from SBUF while the PE is otherwise
    idle - V never round-trips through DRAM.
  * Q chunk 0: one SWDGE cast to SBUF + PE transposes (so the matmul
    stream can start at ~15us).
  * Q chunks 1-3: fp32 loads on the (idle) sync HWDGE ring, engine cast
    to fp16, fp16 write to a DRAM image, then one multi-stripe
    DMA_TRANSPOSE per chunk - all background, off the critical path.
  * E (softmax numerator) transposed SBUF->SBUF per q-tile (no DRAM).
- stage1 emitted in (q-tile, s-chunk) units of 8 matmuls, round-of-4
  interleaved with the V^T transpose chunks so matmuls start as soon as
  the first chunk lands.
- All DMA transposes ride the sync HWDGE ring exclusively (concurrent
  xposes from both HWDGE rings corrupt data).
- All matmuls converted to self-loading (explicit LDWEIGHTS stripped).
"""

import sys

if "/opt/trn_rl_repo" not in sys.path:
    sys.path.insert(0, "/opt/trn_rl_repo")

import numpy as np

B, SQ, SV, D = 8, 2048, 2048, 1024
P = 128
N_CORES = 8


def _strip_all_ldweights(nc):
    """Remove every InstLdweights, migrating its semaphore waits/updates onto
    the next PE instruction (its paired InstMatmult)."""
    import concourse.mybir as mybir

    removed = 0
    for fn in nc.m.functions:
        for bb in fn.blocks:
            keep = []
            pending = []
            for inst in bb.instructions:
                if isinstance(inst, mybir.InstLdweights):
                    if inst.sync_info is not None:
                        pending.append(inst.sync_info)
                    removed += 1
                    continue
                if isinstance(inst, mybir.InstMatmult):
                    inst.ldweights = True
                    if pending:
                        waits, updates = [], []
                        for si in pending:
                            waits.extend(list(si.on_wait))
                            updates.extend(list(si.on_update))
                        mi = inst.sync_info
                        if mi is not None:
                            waits.extend(list(mi.on_wait))
                            updates.extend(list(mi.on_update))
                        inst.sync_info = mybir.SyncInfo(
                            on_wait=waits, on_update=updates
                        )
                        pending = []
                keep.append(inst)
            assert not pending, "dangling LDW sync with no following matmul"
            bb.instructions[:] = keep
    return removed


def build_attention_nc(sq=SQ, sv=SV, d=D, lag=5, rnd=4, compile=True):
    import concourse.bass as bass
    import concourse.mybir as mybir
    from concourse import bacc
    from concourse.tile import TileContext
    from concourse.masks import make_identity

    f32 = mybir.dt.float32
    f16 = mybir.dt.float16
    X = mybir.AxisListType.X
    Exp = mybir.ActivationFunctionType.Exp

    NQT, NST, NKT = sq // P, sv // P, d // P
    CH = 512                  # row chunk (cast/xpose granularity = psum chunk)
    NCH = sv // CH            # 4
    NJ = sv // CH
    DCH = 512
    NDCH = d // DCH
    TPC = CH // P             # tiles per chunk (4)

    nc = bacc.Bacc("TRN2", target_bir_lowering=False, debug=False)
    q = nc.dram_tensor("q", [sq, d], f32, kind="ExternalInput").ap()
    v = nc.dram_tensor("v", [sv, d], f32, kind="ExternalInput").ap()
    out = nc.dram_tensor("out", [sq, d], f32, kind="ExternalOutput").ap()

    with TileContext(nc) as tc:
        with (
            tc.tile_pool(name="dram", bufs=1, space="DRAM") as dram_pool,
            tc.tile_pool(name="resident", bufs=1) as res_pool,
            tc.tile_pool(name="qf32", bufs=1) as qf32_pool,
            tc.tile_pool(name="qf16", bufs=2) as qf16_pool,
            tc.tile_pool(name="ssb", bufs=rnd) as ssb_pool,
            tc.tile_pool(name="esb", bufs=2) as esb_pool,
            tc.tile_pool(name="etp", bufs=lag + 2) as et_pool,
            tc.tile_pool(name="osb", bufs=2) as osb_pool,
            tc.tile_pool(name="small", bufs=3 * (lag + 3)) as small_pool,
            tc.tile_pool(name="xpsum", bufs=2, space="PSUM") as xp_pool,
            tc.tile_pool(name="spsum", bufs=2, space="PSUM") as sp_pool,
            tc.tile_pool(name="opsum", bufs=2, space="PSUM") as opsum_pool,
        ):
            qh = dram_pool.tile([sq, d], f16, name="qh")

            # Residents:
            vf_all = res_pool.tile([P, NST, d], f16, name="vf_all")
            vT_all = res_pool.tile([P, NCH, NKT, CH], f16, name="vT_all")
            qT_all = res_pool.tile([P, NCH, NKT, CH], f16, name="qT_all")
            ident = res_pool.tile([P, P], f16, name="ident")
            make_identity(nc, ident)

            # ---- SWDGE ring (gpsimd): Q chunk-0 cast, then V wave casts ---
            def swdge_cast_chunk(dst_sbuf, src_dram, c):
                src = src_dram[c * CH : (c + 1) * CH, :].rearrange(
                    "(si p) d -> p si d", p=P
                )
                nc.gpsimd.dma_start(out=dst_sbuf, in_=src)

            qf16_c0 = qf16_pool.tile([P, TPC, d], f16, name="qf16")
            swdge_cast_chunk(qf16_c0, q, 0)
            for c in range(NCH):
                swdge_cast_chunk(vf_all[:, c * TPC : (c + 1) * TPC, :], v, c)

            # ---- PE transposes: src [128,128] f16 SBUF -> PSUM f16 --------
            # One PSUM tile gathers all 8 d-stripes of one 128-row tile;
            # a single strided DVE copy scatters them into the resident.
            def pe_xpose_tile(dst_all, c, r, src_tile_ap):
                ps = xp_pool.tile([P, NKT * P], f16, name="xpsum")
                for dk in range(NKT):
                    nc.tensor.transpose(
                        ps[:, dk * P : (dk + 1) * P],
                        src_tile_ap[:, dk * P : (dk + 1) * P],
                        ident,
                    )
                nc.vector.tensor_copy(
                    out=dst_all[:, c, :, r * P : (r + 1) * P],
                    in_=ps.rearrange("p (dk r) -> p dk r", dk=NKT),
                )

            # ---- sync ring: Q c1-3 fp32 loads, qT xposes, eT xposes -------
            qf32_tiles = {}
            for c in range(1, NCH):
                t32 = qf32_pool.tile([P, TPC, d], f32, name="qf32")
                src = q[c * CH : (c + 1) * CH, :].rearrange(
                    "(si p) d -> p si d", p=P
                )
                nc.sync.dma_start(out=t32, in_=src)
                t16 = qf16_pool.tile([P, TPC, d], f16, name="qf16")
                # engine cast fp32 -> fp16 (alternate engines)
                if c % 2 == 1:
                    nc.scalar.copy(t16, t32)
                else:
                    nc.vector.tensor_copy(out=t16, in_=t32)
                # fp16 DRAM image chunk (scalar HWDGE ring)
                dst = qh[c * CH : (c + 1) * CH, :].rearrange(
                    "(si p) d -> p si d", p=P
                )
                nc.scalar.dma_start(out=dst, in_=t16)
                qf32_tiles[c] = t16
            for c in range(1, NCH):
                nc.sync.dma_start_transpose(
                    qT_all[:, c, :, :], qh[c * CH : (c + 1) * CH, :]
                )

            # ---------------- main pipeline --------------------------------
            s_sb = {}
            state = {}
            done = []
            emitted2 = 0

            def unit(qi, j):
                sp = sp_pool.tile([P, CH], f32, name="spsum")
                cq, rq = qi // TPC, qi % TPC
                for dk in range(NKT):
                    nc.tensor.matmul(
                        sp,
                        lhsT=qT_all[:, cq, dk, rq * P : (rq + 1) * P],
                        rhs=vT_all[:, j, dk, :],
                        start=(dk == 0),
                        stop=(dk == NKT - 1),
                    )
                if qi not in s_sb:
                    s_sb[qi] = ssb_pool.tile([P, sv], f32, name="s_sb")
                dst = s_sb[qi][:, j * CH : (j + 1) * CH]
                if j % 2 == 0:
                    nc.scalar.copy(dst, sp)
                else:
                    nc.vector.tensor_copy(out=dst, in_=sp)

            def finish_stage1(qi):
                s = s_sb.pop(qi)
                negm = small_pool.tile([P, 1], f32, name="negm")
                nc.vector.reduce_max(negm, s, axis=X, negate=True)
                e_sb = esb_pool.tile([P, sv], f16, name="e_sb")
                lsum = small_pool.tile([P, 1], f32, name="lsum")
                nc.scalar.activation(
                    e_sb, s, Exp, bias=negm, scale=1.0, accum_out=lsum
                )
                r = small_pool.tile([P, 1], f32, name="r")
                nc.vector.reciprocal(r, lsum)
                eT = et_pool.tile([P, NST, P], f16, name="eT")
                nc.sync.dma_start_transpose(eT, e_sb)
                state[qi] = (r, eT)

            def stage2(qi):
                r, eT = state.pop(qi)
                op = opsum_pool.tile([P, d], f32, name="opsum")
                for sk in range(NST):
                    for c2 in range(NDCH):
                        cs = slice(c2 * DCH, (c2 + 1) * DCH)
                        nc.tensor.matmul(
                            op[:, cs],
                            lhsT=eT[:, sk, :],
                            rhs=vf_all[:, sk, cs],
                            start=(sk == 0),
                            stop=(sk == NST - 1),
                        )
                o_sb = osb_pool.tile([P, d], f32, name="o_sb")
                nc.vector.tensor_scalar_mul(o_sb, op, r)
                nc.scalar.dma_start(
                    out=out[qi * P : (qi + 1) * P, :], in_=o_sb
                )

            # PE program: Q-c0 transposes, then chunk-interleaved stage1.
            for r in range(TPC):
                pe_xpose_tile(qT_all, 0, r, qf16_c0[:, r, :])

            def v_xpose_chunk(c):
                for r in range(TPC):
                    si = c * TPC + r
                    pe_xpose_tile(vT_all, c, r, vf_all[:, si, :])

            ucount = {qi: 0 for qi in range(NQT)}
            for r0 in range(0, NQT, rnd):
                for j in range(NJ):
                    if r0 == 0:
                        v_xpose_chunk(j)
                    for qi in range(r0, r0 + rnd):
                        unit(qi, j)
                        ucount[qi] += 1
                        if ucount[qi] == NJ:
                            finish_stage1(qi)
                            done.append(qi)
                            while len(done) - emitted2 > lag:
                                stage2(done[emitted2])
                                emitted2 += 1
            while emitted2 < len(done):
                stage2(done[emitted2])
                emitted2 += 1

    import os

    if bool(int(os.environ.get("KERNEL_SELF_LDW", "1"))):
        _strip_all_ldweights(nc)
    if compile:
        nc.compile()
    return nc


_CACHE = {}


def _get_nc():
    if "nc" not in _CACHE:
        _CACHE["nc"] = build_attention_nc()
    return _CACHE["nc"]


def _install_trace_support():
    """Synthesize the antenv.axon_hooks module (absent in this image) and
    register the NTFF profile hook + disable the S3 artifact upload."""
    import types
    import antenv

    if "antenv.axon_hooks" not in sys.modules:
        mod = types.ModuleType("antenv.axon_hooks")
        mod._hook = None

        def set_axon_ntff_profile_hook(h):
            mod._hook = h

        def get_axon_ntff_profile_hook():
            return mod._hook

        mod.set_axon_ntff_profile_hook = set_axon_ntff_profile_hook
        mod.get_axon_ntff_profile_hook = get_axon_ntff_profile_hook
        sys.modules["antenv.axon_hooks"] = mod
        antenv.axon_hooks = mod

    mod = sys.modules["antenv.axon_hooks"]
    if mod._hook is None:
        from trn_agent_boot.trn_boot import _ntff_profile_via_ctypes

        mod._hook = _ntff_profile_via_ctypes("/opt/axon/libaxon_pjrt.so")

    import concourse.bass_utils as bu

    bu.upload_artifacts = lambda tmpdir: tmpdir


def kernel(query: np.ndarray, value: np.ndarray) -> np.ndarray:
    from concourse.bass_utils import run_bass_kernel_spmd
    import os

    assert query.shape == (B, SQ, D) and value.shape == (B, SV, D)
    nc = _get_nc()
    in_maps = [
        {
            "q": np.ascontiguousarray(query[b], dtype=np.float32),
            "v": np.ascontiguousarray(value[b], dtype=np.float32),
        }
        for b in range(N_CORES)
    ]
    trace = bool(int(os.environ.get("KERNEL_TRACE", "0")))
    kwargs = {}
    if trace:
        _install_trace_support()
        tdir = os.environ.get("KERNEL_TRACE_DIR")
        if tdir:
            os.makedirs(tdir, exist_ok=True)
            kwargs["tmpdir"] = tdir
    res = run_bass_kernel_spmd(
        nc, in_maps, core_ids=list(range(N_CORES)), trace=trace, **kwargs
    )
    if trace:
        _CACHE["last_results"] = res
    return np.stack([res.results[b]["out"] for b in range(N_CORES)], axis=0)
